# revision 21
# baseline (speedup 1.0000x reference)
"""Trainium2 Bass kernel: causal MHA block (B=2, S=2048, D=4096, 32 heads x 128,
fp32 I/O, interleaved RoPE), tensor-parallel over heads on 8 NeuronCores with a
per-batch AllToAll into a sequence-parallel output projection.

v2 (vs. the fp32r baseline): all PE-facing data is bf16 (same PE rate, half the
HBM bytes and half the DVE/ACT element time), Q/K/V are computed in a single
pass over x with all three weights SBUF-resident, wo is read once (not once per
batch) and prefetched during attention, every DMA is a large merged transfer
(16-32KB per partition row) to kill SWDGE issue overhead, and the causal mask
is a multiplicative 0/1 bf16 mask applied after exp.

Per core i (4 heads = 512 features):
  QKV    : per 512-token block: xq^T/xk^T feature-major chains (RoPE fused via
           pair-swap permutation matmul + DVE combine), v token-major chains.
  attn   : per (batch, head): K-stationary transposed scores, exp on ScalarE
           (PSUM->bf16), denominator via a ones-matmul chain, PV chain,
           normalize with a reciprocal broadcast matmul.
  A2A    : per batch, head-slices -> token-slices across 8 cores (bf16).
  WO     : out[tok_slice] = attn @ wo^T, one pass over wo for both batches.
"""

import sys

if "/opt/trn_rl_repo" not in sys.path:
    sys.path.insert(0, "/opt/trn_rl_repo")

import numpy as np

import concourse.bass as bass
import concourse.tile as tile
from concourse import bacc, mybir
from concourse.bass_utils import run_bass_kernel_spmd

F32 = mybir.dt.float32
BF16 = mybir.dt.bfloat16

B, S, D = 2, 2048, 4096
H, HD = 32, 128
NCORES = 8
HPC = H // NCORES        # heads per core
F = HPC * HD             # 512 features per core
TOK = B * S              # 4096 tokens
KT = D // 128            # 32 contraction tiles
NB = TOK // 512          # 8 token blocks of 512
SCALE = 1.0 / float(np.sqrt(HD))

_CACHE = {}


def _build():
    nc = bacc.Bacc("TRN2", target_bir_lowering=False, debug=False,
                   num_devices=NCORES)

    x_d = nc.dram_tensor("xt", [NB, 2, 128, 16 * 512], BF16,
                         kind="ExternalInput")
    wq_d = nc.dram_tensor("wqT", [128, KT * F], BF16, kind="ExternalInput")
    wk_d = nc.dram_tensor("wkT", [128, KT * F], BF16, kind="ExternalInput")
    wv_d = nc.dram_tensor("wvT", [128, KT * F], BF16, kind="ExternalInput")
    wo_d = nc.dram_tensor("woT", [D // 512, 128, KT * 512], BF16,
                          kind="ExternalInput")
    cos_d = nc.dram_tensor("cosE", [128, S], BF16, kind="ExternalInput")
    sin_d = nc.dram_tensor("sinE", [128, S], BF16, kind="ExternalInput")
    tri_d = nc.dram_tensor("tri01", [128, 4 * 512], BF16, kind="ExternalInput")
    perm_d = nc.dram_tensor("permT", [128, 128], BF16, kind="ExternalInput")
    ones_d = nc.dram_tensor("ones", [128, 128], BF16, kind="ExternalInput")
    out_d = nc.dram_tensor("out", [TOK // NCORES, D], F32,
                           kind="ExternalOutput")

    with tile.TileContext(nc) as tc:
        dram = tc.alloc_tile_pool(name="dram", bufs=1, space="DRAM")
        q_sp = [dram.tile([HPC, 128, S], BF16, name=f"q_sp{b}")
                for b in range(B)]
        k_sp = [dram.tile([HPC, 128, S], BF16, name=f"k_sp{b}")
                for b in range(B)]
        v_sp = [dram.tile([128, (S // 128) * F], BF16, name=f"v_sp{b}")
                for b in range(B)]
        a2a_in = [dram.tile([NCORES, F, 256], BF16, name="a2a_in0"),
                  dram.tile([NCORES, F // 2, 256], BF16, name="a2a_in1a"),
                  dram.tile([NCORES, F // 2, 256], BF16, name="a2a_in1b")]
        a2a_out = [dram.tile([NCORES, F, 256], BF16, name="a2a_out0"),
                   dram.tile([NCORES, F // 2, 256], BF16, name="a2a_out1a"),
                   dram.tile([NCORES, F // 2, 256], BF16, name="a2a_out1b")]

        with tc.tile_pool(name="consts", bufs=1) as cpool:
            perm_sb = cpool.tile([128, 128], BF16)
            nc.sync.dma_start(out=perm_sb[:], in_=perm_d[:, :])
            ones_sb = cpool.tile([128, 128], BF16)
            nc.sync.dma_start(out=ones_sb[:], in_=ones_d[:, :])
            cos_sb = cpool.tile([128, S], BF16)
            nc.sync.dma_start(out=cos_sb[:], in_=cos_d[:, :])
            sin_sb = cpool.tile([128, S], BF16)
            nc.sync.dma_start(out=sin_sb[:], in_=sin_d[:, :])
            tri_sb = cpool.tile([128, 4 * 512], BF16)
            nc.sync.dma_start(out=tri_sb[:], in_=tri_d[:, :])

            # ======== single pass over x: Q, K (feature-major + RoPE) and V
            # (token-major), all three weights SBUF-resident in bf16
            with tc.tile_pool(name="wpool", bufs=1) as wpool, \
                 tc.tile_pool(name="xpool", bufs=2) as xpool, \
                 tc.tile_pool(name="qkvw", bufs=2) as work, \
                 tc.tile_pool(name="prps", bufs=2, space="PSUM") as prps, \
                 tc.tile_pool(name="rotps", bufs=2, space="PSUM") as rotps:

                w_sb = {}
                half_c = KT * F // 2
                for nm, w_d in (("q", wq_d), ("k", wk_d), ("v", wv_d)):
                    # lo/hi separate tiles: the first 16 contraction tiles of
                    # the first chains only wait on the lo half (half the
                    # bytes) instead of the whole weight
                    lo = wpool.tile([128, half_c], BF16, tag=f"w{nm}l",
                                    name=f"w{nm}l")
                    hi = wpool.tile([128, half_c], BF16, tag=f"w{nm}h",
                                    name=f"w{nm}h")
                    if nm == "q":
                        # the first QK chain waits on wq: force it to the
                        # front of both DMA queues (the scheduler orders
                        # zero-dep DMAs arbitrarily otherwise)
                        with tc.high_priority():
                            nc.sync.dma_start(out=lo[:], in_=w_d[:, :half_c])
                            nc.scalar.dma_start(out=hi[:], in_=w_d[:, half_c:])
                    else:
                        nc.sync.dma_start(out=lo[:], in_=w_d[:, :half_c])
                        nc.scalar.dma_start(out=hi[:], in_=w_d[:, half_c:])
                    w_sb[nm] = (lo, hi)

                for nb in range(NB):
                    xhs = []
                    for half in range(2):
                        xt = xpool.tile([128, half_c], BF16, tag=f"x{half}",
                                        name=f"x{half}")
                        nc.gpsimd.dma_start(out=xt[:], in_=x_d[nb, half, :, :])
                        xhs.append(xt)
                    pos = (nb % (S // 512)) * 512
                    for m in range(2 * HPC):
                        wt = w_sb["q"] if m < HPC else w_sb["k"]
                        o_sp = q_sp if m < HPC else k_sp
                        h = m % HPC
                        ps = prps.tile([128, 512], F32, name="ps")
                        for kt in range(KT):
                            hf, kk = divmod(kt, 16)
                            nc.tensor.matmul(
                                ps[:],
                                wt[hf][:, kk * F + h * 128:
                                       kk * F + (h + 1) * 128],
                                xhs[hf][:, kk * 512:(kk + 1) * 512],
                                start=(kt == 0), stop=(kt == KT - 1))
                        raw = work.tile([128, 512], BF16, tag="raw",
                                        name="raw")
                        nc.scalar.copy(raw[:], ps[:])
                        rot = rotps.tile([128, 512], F32, name="rot")
                        nc.tensor.matmul(rot[:], perm_sb[:], raw[:],
                                         start=True, stop=True)
                        t1 = work.tile([128, 512], F32, tag="t1", name="t1")
                        nc.vector.tensor_mul(t1[:], raw[:],
                                             cos_sb[:, pos:pos + 512])
                        t2 = work.tile([128, 512], F32, tag="t2", name="t2")
                        nc.vector.tensor_mul(t2[:], rot[:],
                                             sin_sb[:, pos:pos + 512])
                        qf = work.tile([128, 512], BF16, tag="qf", name="qf")
                        nc.vector.tensor_add(qf[:], t1[:], t2[:])
                        nc.sync.dma_start(
                            out=o_sp[nb // 4][h, :,
                                              (nb % 4) * 512:
                                              (nb % 4 + 1) * 512],
                            in_=qf[:])
                    for ts in range(4):
                        ps = prps.tile([128, 512], F32, name="psv")
                        for kt in range(KT):
                            hf, kk = divmod(kt, 16)
                            nc.tensor.matmul(
                                ps[:],
                                xhs[hf][:, kk * 512 + ts * 128:
                                        kk * 512 + (ts + 1) * 128],
                                w_sb["v"][hf][:, kk * F:(kk + 1) * F],
                                start=(kt == 0), stop=(kt == KT - 1))
                        vf = work.tile([128, 512], BF16, tag="vf", name="vf")
                        nc.scalar.copy(vf[:], ps[:])
                        st_g = nb * 4 + ts
                        nc.sync.dma_start(
                            out=v_sp[st_g // 16][:,
                                                 (st_g % 16) * F:
                                                 (st_g % 16 + 1) * F],
                            in_=vf[:])

            # ======== attention + per-batch AllToAll, then single-pass WO
            with tc.tile_pool(name="aqk", bufs=2) as apool, \
                 tc.tile_pool(name="avp", bufs=2) as vpool, \
                 tc.tile_pool(name="exw", bufs=26) as expool, \
                 tc.tile_pool(name="amisc", bufs=2) as misc, \
                 tc.tile_pool(name="atsb", bufs=1) as atsb, \
                 tc.tile_pool(name="wop", bufs=2) as wopool, \
                 tc.tile_pool(name="wout", bufs=3) as wout:

                # wo n=0,1 prefetched on sync, pinned (via the scheduler's
                # model clock) to mid-attention: early enough to hide the
                # transfer, late enough not to crowd the weight/x streams or
                # the collectives
                wo_tiles = {}
                for n in range(2):
                    t = wopool.tile([128, KT * 512], BF16, tag="wo",
                                    name="wo_sb")
                    with tc.tile_wait_until(0.78 + 0.04 * n):
                        nc.sync.dma_start(out=t[:], in_=wo_d[n, :, :])
                    wo_tiles[n] = t

                at_sb = [None, None]

                with tc.tile_pool(name="scps", bufs=4, space="PSUM") as scps, \
                     tc.tile_pool(name="pvps", bufs=2, space="PSUM") as pvps, \
                     tc.tile_pool(name="dps", bufs=1, space="PSUM") as dps, \
                     tc.tile_pool(name="bcps", bufs=1, space="PSUM") as bcps:

                    def finish_gen(b, h, qt, exs, v_sb):
                        # dsum/pv chains as a generator: yields after each
                        # matmul so the caller can interleave them between
                        # the next qt's score matmuls (keeps PE fed while
                        # ScalarE works through the exp backlog)
                        nkt = 4 * qt + 4
                        dsum = dps.tile([1, 512], F32, name="dsum",
                                        tag="dsum")
                        for kt in range(nkt):
                            nc.tensor.matmul(
                                dsum[:], ones_sb[:, 0:1], exs[kt][:],
                                start=(kt == 0), stop=(kt == nkt - 1))
                            yield
                        rec32 = misc.tile([1, 512], F32, tag="rec32",
                                          name="rec32")
                        nc.vector.reciprocal_approx_fast(out=rec32[:],
                                                         in_=dsum[:])
                        rec = misc.tile([1, 512], BF16, tag="rec", name="rec")
                        with nc.allow_low_precision(
                                reason="1/denom feeds bf16 matmul"):
                            nc.vector.tensor_copy(rec[:], rec32[:])
                        pv = pvps.tile([128, 512], F32, name="pv")
                        for kt in range(nkt):
                            nc.tensor.matmul(
                                pv[:],
                                v_sb[:, kt * F + h * 128:
                                     kt * F + (h + 1) * 128],
                                exs[kt][:],
                                start=(kt == 0), stop=(kt == nkt - 1))
                            yield
                        # a few spare yields so the bc matmul (which waits on
                        # the DVE reciprocal) lands well after it completes
                        for _ in range(5):
                            yield
                        bc = bcps.tile([128, 512], F32, name="bc")
                        nc.tensor.matmul(bc[:], ones_sb[0:1, :], rec[:],
                                         start=True, stop=True)
                        bc_sb = misc.tile([128, 512], BF16, tag="bcsb",
                                          name="bc_sb")
                        nc.vector.tensor_copy(bc_sb[:], bc[:])
                        at = misc.tile([128, 512], BF16, tag="at", name="at")
                        nc.vector.tensor_mul(at[:], pv[:], bc_sb[:])
                        tgt = a2a_in[0] if b == 0 else a2a_in[1 + h // 2]
                        row = (h if b == 0 else h % 2) * 128
                        for u in range(2):
                            nc.sync.dma_start(
                                out=tgt[2 * qt + u, row:row + 128, :],
                                in_=at[:, u * 256:(u + 1) * 256])

                    for b in range(B):
                        # batch-0 attention inputs load on the (idle) gpsimd
                        # queue as soon as the QKV spills land; batch-1's go
                        # on scalar because gpsimd is blocked by the first
                        # collective by then
                        ld = nc.gpsimd if b == 0 else nc.scalar
                        v_sb = vpool.tile([128, (S // 128) * F], BF16,
                                          tag="v", name="v_sb")
                        ld.dma_start(out=v_sb[:], in_=v_sp[b][:, :])
                        pending = None
                        for h in range(HPC):
                            q_sb = apool.tile([128, S], BF16, tag="q",
                                              name="q_sb")
                            ld.dma_start(
                                out=q_sb[:], in_=q_sp[b][h, :, :])
                            k_sb = apool.tile([128, S], BF16, tag="k",
                                              name="k_sb")
                            ld.dma_start(
                                out=k_sb[:], in_=k_sp[b][h, :, :])

                            for qt in range(4):
                                nkt = 4 * qt + 4
                                exs = []
                                for kt in range(nkt):
                                    sc = scps.tile([128, 512], F32, name="sc")
                                    nc.tensor.matmul(
                                        sc[:],
                                        k_sb[:, kt * 128:(kt + 1) * 128],
                                        q_sb[:, qt * 512:(qt + 1) * 512],
                                        start=True, stop=True)
                                    ex = expool.tile([128, 512], BF16,
                                                     tag="ex", name="ex")
                                    nc.scalar.activation(
                                        ex[:], sc[:],
                                        mybir.ActivationFunctionType.Exp,
                                        scale=SCALE)
                                    r = kt - 4 * qt
                                    if r >= 0:
                                        exm = expool.tile(
                                            [128, 512], BF16, tag="ex",
                                            name="exm")
                                        nc.vector.tensor_mul(
                                            exm[:], ex[:],
                                            tri_sb[:, r * 512:(r + 1) * 512])
                                        ex = exm
                                    exs.append(ex)
                                    if pending is not None:
                                        for _ in range(2):
                                            if next(pending, "end") == "end":
                                                pending = None
                                                break
                                if pending is not None:
                                    for _ in pending:
                                        pass
                                pending = finish_gen(b, h, qt, exs, v_sb)
                            if b == 1 and h == 1:
                                # half-A2A for batch 1 (heads 0-1 of every
                                # core) fired mid-batch so WO's accumulation
                                # can start on these features while the
                                # second half is still in flight
                                for _ in pending:
                                    pass
                                pending = None
                                nc.gpsimd.collective_compute(
                                    "AllToAll", mybir.AluOpType.bypass,
                                    replica_groups=[list(range(NCORES))],
                                    ins=[a2a_in[1][:]], outs=[a2a_out[1][:]])
                                t = atsb.tile([128, KT * 256], BF16,
                                              tag="at1", name="at_sb1")
                                for g in range(2):
                                    nc.gpsimd.dma_start(
                                        out=t[:, g * 2048:(g + 1) * 2048]
                                        .rearrange("p (j t) -> p j t", j=8),
                                        in_=a2a_out[1][:, g * 128:
                                                       (g + 1) * 128, :]
                                        .rearrange("j p t -> p j t"))
                                at_sb[1] = t
                        if pending is not None:
                            for _ in pending:
                                pass
                        if b == 0:
                            nc.gpsimd.collective_compute(
                                "AllToAll", mybir.AluOpType.bypass,
                                replica_groups=[list(range(NCORES))],
                                ins=[a2a_in[0][:]], outs=[a2a_out[0][:]])
                            t = atsb.tile([128, KT * 256], BF16, tag="at0",
                                          name="at_sb0")
                            for g in range(4):
                                nc.gpsimd.dma_start(
                                    out=t[:, g * 2048:(g + 1) * 2048]
                                    .rearrange("p (j t) -> p j t", j=8),
                                    in_=a2a_out[0][:, g * 128:(g + 1) * 128, :]
                                    .rearrange("j p t -> p j t"))
                            at_sb[0] = t
                        else:
                            nc.gpsimd.collective_compute(
                                "AllToAll", mybir.AluOpType.bypass,
                                replica_groups=[list(range(NCORES))],
                                ins=[a2a_in[2][:]], outs=[a2a_out[2][:]])
                            for g in range(2, 4):
                                nc.gpsimd.dma_start(
                                    out=at_sb[1][:, g * 2048:(g + 1) * 2048]
                                    .rearrange("p (j t) -> p j t", j=8),
                                    in_=a2a_out[2][:, (g - 2) * 128:
                                                   (g - 1) * 128, :]
                                    .rearrange("j p t -> p j t"))

                    # ---- WO inside the same PSUM scope (pss reuses the
                    # "sc" tag ring) so no pool-transition barrier separates
                    # attention from the output projection.
                    # feature tiles in halves order: g 0-1 (delivered by the
                    # first half-A2A of batch 1) before g 2-3, so batch-1
                    # chains can begin before the second half lands
                    tile_order = ([(j, g) for g in (0, 1) for j in range(8)]
                                  + [(j, g) for g in (2, 3) for j in range(8)])

                    def wo_chain(n, b, wo_sb):
                        pss = [scps.tile([128, 512], F32, tag="sc",
                                         name="psw") for mt in range(2)]
                        for idx, (j, g) in enumerate(tile_order):
                            kt = j * 4 + g
                            col = g * 2048 + j * 256
                            for mt in range(2):
                                nc.tensor.matmul(
                                    pss[mt][:],
                                    at_sb[b][:, col + mt * 128:
                                             col + (mt + 1) * 128],
                                    wo_sb[:, kt * 512:(kt + 1) * 512],
                                    start=(idx == 0), stop=(idx == KT - 1))
                        for mt in range(2):
                            o_sb = wout.tile([128, 512], F32, name="o_sb")
                            nc.scalar.copy(o_sb[:], pss[mt][:])
                            nc.sync.dma_start(
                                out=out_d[b * 256 + mt * 128:
                                          b * 256 + (mt + 1) * 128,
                                          n * 512:(n + 1) * 512],
                                in_=o_sb[:])

                    def get_wo(n):
                        if n in wo_tiles:
                            return wo_tiles.pop(n)
                        t = wopool.tile([128, KT * 512], BF16, tag="wo",
                                        name="wo_sb")
                        nc.sync.dma_start(out=t[:], in_=wo_d[n, :, :])
                        return t

                    # batch-0 chains for n=0,1 first: they only need the
                    # (long-finished) first A2A and run while batch-1's
                    # collectives drain
                    order = [(0, 0), (1, 0), (0, 1), (1, 1)]
                    for n in range(2, D // 512):
                        order += [(n, 0), (n, 1)]
                    live = {}
                    for n, b in order:
                        if n not in live:
                            live[n] = get_wo(n)
                        wo_chain(n, b, live[n])
                        if b == 1:
                            del live[n]

    nc.compile()
    return nc


def _host_inputs(x, wq, wk, wv, wo):
    import ml_dtypes
    BF = ml_dtypes.bfloat16

    x = np.asarray(x, dtype=np.float32).reshape(TOK, D)
    # xt[nb, half, p, kk*512+t] = x[nb*512+t, half*2048+kk*128+p]
    xt = np.ascontiguousarray(
        x.T.reshape(2, 16, 128, NB, 512).transpose(3, 0, 2, 1, 4)
        .reshape(NB, 2, 128, 16 * 512)).astype(BF)

    # woT[n, p, kt*512+o] = wo[n*512+o, kt*128+p]
    wot = np.ascontiguousarray(
        np.asarray(wo, np.float32).T.reshape(KT, 128, D // 512, 512)
        .transpose(2, 1, 0, 3).reshape(D // 512, 128, KT * 512)).astype(BF)

    inv = (1.0 / (10000.0 ** (np.arange(0, HD, 2, dtype=np.float64) / HD)))
    fr = np.outer(np.arange(S, dtype=np.float64), inv)        # [S, HD/2]
    cosE = np.repeat(np.cos(fr).T, 2, axis=0).astype(BF)      # [128, S]
    sinE = np.repeat(np.sin(fr).T, 2, axis=0).astype(BF)

    # tri01[p, r*512+q] = 1 where r*128+p <= q (causal keep), else 0
    tri = np.zeros([128, 4 * 512], dtype=np.float32)
    qi = np.arange(512)
    pi = np.arange(128)
    for r in range(4):
        tri[:, r * 512:(r + 1) * 512][
            (r * 128 + pi)[:, None] <= qi[None, :]] = 1.0
    tri = tri.astype(BF)

    permT = np.zeros([128, 128], dtype=np.float32)
    ii = np.arange(0, 128, 2)
    permT[ii + 1, ii] = -1.0
    permT[ii, ii + 1] = 1.0
    permT = permT.astype(BF)

    ones = np.ones([128, 128], dtype=BF)

    def wtile(w, i):
        # [p, kt*512+f] = w[i*512+f, kt*128+p]
        sl = np.asarray(w, np.float32)[i * F:(i + 1) * F, :]
        return np.ascontiguousarray(
            sl.T.reshape(KT, 128, F).transpose(1, 0, 2)
            .reshape(128, KT * F)).astype(BF)

    maps = []
    for i in range(NCORES):
        maps.append(dict(
            xt=xt,
            wqT=wtile(wq, i), wkT=wtile(wk, i), wvT=wtile(wv, i),
            woT=wot, cosE=cosE, sinE=sinE, tri01=tri, permT=permT,
            ones=ones,
        ))
    return maps


def kernel(x, start_pos, wq, wk, wv, wo, _trace=False):
    if "nc" not in _CACHE:
        _CACHE["nc"] = _build()
    nc = _CACHE["nc"]
    maps = _host_inputs(x, wq, wk, wv, wo)
    res = run_bass_kernel_spmd(nc, maps, core_ids=list(range(NCORES)),
                               trace=_trace)
    _CACHE["last"] = res
    full = np.empty([TOK, D], dtype=np.float32)
    for j in range(NCORES):
        o = res.results[j]["out"]
        full[j * 256:(j + 1) * 256] = o[:256]
        full[S + j * 256: S + (j + 1) * 256] = o[256:]
    return full.reshape(B, S, D)
